# revision 16
# baseline (speedup 1.0000x reference)
"""Trainium2 Bass kernel for nn_Attention_layer (GNN message passing attention).

Math (see harness reference):
  x_Q = [input_x, pe_Q]  (N, 1024);  x_K = [input_x, pe_K]
  Q = x_Q @ WQ[h] + qb;  K = x_K @ WK[h] + kb;  V = input_x @ WV[h] + vb
  attn = softmax(Q K^T / 16, axis=k);  out = concat_h(attn @ V) @ lin_w.T + lin_b

Distribution: 8 NeuronCores, query-dim (N) sharded 512 rows/core; K/V work
replicated (no collectives).  Everything is computed in the transposed domain
(scores^T with k-nodes on partitions) so no on-device transposes are needed.

v2+ design (from perfetto analysis of v1, which was ScalarE-bound at ~80%
occupancy on 16.8M exp elements/core; v1 = 180-211us, this = ~172us in the
same device state):
  - exp is SPLIT between ScalarE (exact, heads 4m+{0,1}) and VectorE
    (Schraudolph bit-trick exp, heads 4m+{2,3}): one tensor_scalar mult-add
    per tile whose f32->int16 convert emits exactly the BF16 bit pattern of
    exp(s/16).  End-to-end rel err 9.8e-3 (gate 2e-2).
  - scores live in 4 one-bank PSUM tiles (tags st0..st3), one per head, so
    each exp call releases its bank independently (finer WAR pipelining;
    2 ACT calls + 2 DVE calls per group).
  - PV and Z (ones-row) matmuls accumulate IN PSUM across all 32 k-chunks
    (start/stop chains) instead of VectorE adds - frees ~106us/core of DVE.
  - head-group-OUTER loop (mg=0 fully, then mg=1) so only one PV + one Z
    accumulator bank is live at a time; PSUM budget: 4x scores + pv + z +
    2x proj work ring = 8 banks exactly.  Per-mg normalization chain runs
    right after each mg's drain (mg0's hides inside the mg1 phase).
  - projections (Q/K/V) interleaved into the main loop via the work ring,
    paced a bit ahead of the attention groups that consume them; input DMA
    split across BOTH hwdge queues (sync + scalar) to double head bandwidth.

Measured laws this design is built around (this hw):
  - a 4-way tiled matmul group (row- or col-tiled) streams ~2 cols/cycle
    aggregate; serial full-array MMs get ~1 col/cycle (216ns per N=512).
  - ACTIVATE [128,512] PSUM->SBUF ~687ns; VectorE tensor_scalar ~682ns.
  - do NOT interleave MMs of two full-array accumulation chains (alternating
    LDWEIGHTS at the same tile position corrupts results silently).
  - fp8 (e4m3) projections fail the 2e-2 gate (measured 4-5e-2 end to end).
"""

import os
import sys
import numpy as np
import ml_dtypes

for _p in ("/opt/trn_rl_repo", "/root/.axon_site/_ro/trn_rl_repo"):
    if os.path.isdir(_p) and _p not in sys.path:
        sys.path.insert(0, _p)

N = 4096
IND = 256          # input_x dim
QKD = 1024         # concat dim for Q/K projections
H = 8              # heads
HD = 32            # head dim
HID = 256          # H * HD
NCORES = 8
NQ = N // NCORES   # 512 query rows per core
SCALE = 1.0 / 16.0  # 1/sqrt(HID)

# Schraudolph exp constants (see numpy calibration): the top 16 bits of the
# f32 pattern of exp(s/16) are int16(s * A_EXP + B_EXP) -- i.e. one VectorE
# tensor_scalar writing int16 produces the BF16 of exp.  C=366000 minimizes
# max rel err (~3.3%).
A_EXP = float((2.0 ** 23) * np.log2(np.e) / 16.0 / 65536.0)
B_EXP = float((127.0 * 2.0 ** 23 - 366000.0) / 65536.0)

_CACHE = {}


def _build_nc():
    from contextlib import ExitStack
    import concourse.bacc as bacc
    import concourse.tile as tile
    import concourse.mybir as mybir
    from concourse.bass import ds, ts

    f32 = mybir.dt.float32
    i16 = mybir.dt.int16
    bf16 = mybir.dt.bfloat16
    Exp = mybir.ActivationFunctionType.Exp
    Ident = mybir.ActivationFunctionType.Identity
    Copy = mybir.ActivationFunctionType.Copy
    mult = mybir.AluOpType.mult
    add = mybir.AluOpType.add

    nc = bacc.Bacc("TRN2", target_bir_lowering=False, debug=False,
                   num_devices=NCORES)

    # ---- DRAM I/O (per-core shards prepared on host) ----
    xkT = nc.dram_tensor("xkT", [QKD, N], bf16, kind="ExternalInput")   # [x;peK]^T
    xqT = nc.dram_tensor("xqT", [QKD, NQ], bf16, kind="ExternalInput")  # [x;peQ]^T rows blk
    wq = nc.dram_tensor("wq", [QKD, HID], bf16, kind="ExternalInput")   # [d,(h,hd)]
    wk = nc.dram_tensor("wk", [QKD, HID], bf16, kind="ExternalInput")
    wv = nc.dram_tensor("wv", [IND, HID], bf16, kind="ExternalInput")
    lwT = nc.dram_tensor("lwT", [HID, HID], bf16, kind="ExternalInput")  # lin_w.T
    bias4 = nc.dram_tensor("bias4", [128, 8], f32, kind="ExternalInput")  # [p, 4m+i]
    out = nc.dram_tensor("out", [HID, NQ], f32, kind="ExternalOutput")   # out^T

    # Z-row gather: zacc drains hold Z_{4mg+r} at partition 32r; gather to
    # rows 0..3 (mg-independent).
    selz_np = np.zeros((128, 4), dtype=np.float32)
    for r in range(4):
        selz_np[32 * r, r] = 1.0
    selz_dram = nc.inline_tensor(np.ascontiguousarray(selz_np), name="selz_const")
    # 1/Z broadcast: psb[32j+hd, q] = zrm[j, q]  (mg-independent)
    bsel_np = np.zeros((4, 128), dtype=np.float32)
    for j in range(4):
        bsel_np[j, 32 * j:32 * j + 32] = 1.0
    bsel_dram = nc.inline_tensor(bsel_np, name="bsel_const")
    ones_np = np.ones((128, 1), dtype=ml_dtypes.bfloat16)
    ones_dram = nc.inline_tensor(ones_np, name="ones_const")

    with tile.TileContext(nc) as tc, ExitStack() as ctx:
        consts = ctx.enter_context(tc.tile_pool(name="consts", bufs=1))
        big = ctx.enter_context(tc.tile_pool(name="big", bufs=1))
        ptp = ctx.enter_context(tc.tile_pool(name="ptp", bufs=3))
        stp = ctx.enter_context(tc.tile_pool(name="stp", bufs=1, space="PSUM"))

        # ---- SBUF tiles ----
        xkt = big.tile([128, 8, N], bf16, tag="xkt")       # x_K^T  (8 c-chunks)
        xqt = big.tile([128, 8, NQ], bf16, tag="xqt")      # x_Q^T block
        wqt = consts.tile([128, 8, HID], bf16, tag="wqt")
        wkt = consts.tile([128, 8, HID], bf16, tag="wkt")
        wvt = consts.tile([128, 2, HID], bf16, tag="wvt")
        lwt = consts.tile([128, 2, HID], bf16, tag="lwt")
        bt = consts.tile([128, 8], f32, tag="bt")          # [p, 4m+i]
        selz = consts.tile([128, 4], f32, tag="selz")
        bsel = consts.tile([4, 128], f32, tag="bsel")
        ones = consts.tile([128, 1], bf16, tag="ones")

        kt = big.tile([128, 2, N], bf16, tag="kt")         # K^T rows (h,hd)
        qt = big.tile([128, 2, NQ], bf16, tag="qt")        # Q^T
        vt = big.tile([128, 32, HID], bf16, tag="vt")      # V node-major
        pvs = big.tile([128, 2, NQ], f32, tag="pvs")       # PV accum drains
        zsb = big.tile([128, 2, NQ], f32, tag="zsb")       # Z accum drains
        zrm = big.tile([4, 2, NQ], f32, tag="zrm")         # 1/Z per head (mg slot)
        attn = big.tile([128, 2, NQ], bf16, tag="attn")    # normalized attn_x^T
        outsb = big.tile([128, 2, NQ], f32, tag="outsb")

        # ---- const / weight DMAs, ordered by first consumer ----
        xkT_r = xkT.rearrange("(c p) (n q) -> n p c q", p=128, q=512)
        xqT_r = xqT.rearrange("(c p) q -> p c q", p=128)
        # scalar's HWDGE queue carries ONLY what the first attention group
        # needs (so the first exp isn't stuck behind bulk-DMA doorbells on the
        # ScalarE queue); sync streams the rest, staying ahead of the k-chunk
        # consumption rate.
        nc.sync.dma_start(wqt[:], wq.rearrange("(c p) o -> p c o", p=128))
        nc.sync.dma_start(wkt[:], wk.rearrange("(c p) o -> p c o", p=128))
        nc.scalar.dma_start(bt[:], bias4[:])
        nc.scalar.dma_start(xqt[:, :4], xqT_r[:, :4])
        nc.scalar.dma_start(xqt[:, 4:], xqT_r[:, 4:])
        nc.scalar.dma_start(xkt[:, :, ds(0, 128)], xkT_r[0][:, :, ds(0, 128)])
        nc.scalar.dma_start(xkt[:, :, ds(128, 384)], xkT_r[0][:, :, ds(128, 384)])
        nc.scalar.dma_start(wvt[:], wv.rearrange("(c p) o -> p c o", p=128))
        nc.scalar.dma_start(lwt[:], lwT.rearrange("(c p) o -> p c o", p=128))
        nc.scalar.dma_start(selz[:], selz_dram[:])
        nc.scalar.dma_start(bsel[:], bsel_dram[:])
        nc.scalar.dma_start(ones[:], ones_dram[:])
        for n in range(1, 8):
            nc.sync.dma_start(xkt[:, :, ts(n, 512)], xkT_r[n])

        # preload the ACT exp table set while DMAs land
        actwarm = consts.tile([8, 16], f32, tag="actwarm")
        nc.vector.memset(actwarm[:], 0.0)
        nc.scalar.activation(actwarm[:], actwarm[:], Exp)

        # ---- projection units (PSUM work ring, 2 banks) ----
        # Two independent half-units interleave their MMs so the PE streams
        # 2-wide (adjacent MMs hit different banks; serial same-bank chains
        # only reach 1 col/cycle).
        def qk_proj_pair(specs):
            # specs: list of 2 (kind, n, m) where kind in 'q','k'
            tiles = []
            for kind, n, m in specs:
                ps = stp.tile([128, NQ], f32, tag="work", bufs=2,
                              name=f"{kind}p{n}_{m}")
                tiles.append(ps)
            for (kind, n, m), ps in zip(specs, tiles):
                for c in range(8):
                    if kind == 'q':
                        nc.tensor.matmul(ps[:, :NQ], wqt[:, c, ts(m, 128)],
                                         xqt[:, c, :],
                                         start=(c == 0), stop=(c == 7))
                    else:
                        nc.tensor.matmul(ps[:, :NQ], wkt[:, c, ts(m, 128)],
                                         xkt[:, c, ts(n, 512)],
                                         start=(c == 0), stop=(c == 7))
            for (kind, n, m), ps in zip(specs, tiles):
                if kind == 'q':
                    nc.vector.tensor_scalar_add(qt[:, m, :], ps[:, :NQ],
                                                bt[:, 4 * m + 0:4 * m + 1])
                else:
                    nc.vector.tensor_scalar_add(kt[:, m, ts(n, 512)], ps[:, :NQ],
                                                bt[:, 4 * m + 1:4 * m + 2])

        def k_proj_unit(n, m, lo=0, w=512):
            ps = stp.tile([128, NQ], f32, tag="work", bufs=2, name=f"kp{n}_{m}")
            for c in range(8):
                nc.tensor.matmul(ps[:, :w], wkt[:, c, ts(m, 128)],
                                 xkt[:, c, ds(512 * n + lo, w)],
                                 start=(c == 0), stop=(c == 7))
            nc.vector.tensor_scalar_add(kt[:, m, ds(512 * n + lo, w)], ps[:, :w],
                                        bt[:, 4 * m + 1:4 * m + 2])

        def v_proj_unit(g):
            # covers node chunks 2g, 2g+1 -> vt[:, 2g:2g+2, :]; the two
            # chunks' accumulations interleave for 2-wide streaming.
            ps = stp.tile([128, NQ], f32, tag="work", bufs=2, name=f"vp{g}")
            for kc in (2 * g, 2 * g + 1):
                off = 256 * (kc - 2 * g)
                for c in range(2):
                    nc.tensor.matmul(ps[:, ds(off, HID)],
                                     xkt[:, c, ds(128 * kc, 128)],
                                     wvt[:, c, :], start=(c == 0), stop=(c == 1))
            nc.vector.tensor_copy(out=vt[:, 2 * g:2 * g + 2, :],
                                  in_=ps[:].rearrange("p (g o) -> p g o", o=HID))

        # ---- prologue: minimum for attention group (kc=0, mg=0) ----
        qk_proj_pair([('q', 0, 0), ('k', 0, 0)])
        v_proj_unit(0)

        # remaining proj work, scheduled into the group stream.
        # group index g = 32*mg + kc  (mg outer).  A k_proj_unit(n, m) must
        # complete before group (32*m + 4*n); v_proj_unit(g) before groups
        # (mg, 2g) i.e. min(2g, 32+2g)=2g for mg=0... v needed by both mgs at
        # kc=2g -> before group 2g.
        pre_work = {}

        def sched(slot, fn):
            pre_work.setdefault(max(0, slot), []).append(fn)

        # mg=0 K units paired with q(1)/each other; needed at groups 4n.
        sched(0, lambda: qk_proj_pair([('k', 1, 0), ('q', 0, 1)]))
        sched(3, lambda: qk_proj_pair([('k', 2, 0), ('k', 3, 0)]))
        sched(9, lambda: qk_proj_pair([('k', 4, 0), ('k', 5, 0)]))
        sched(17, lambda: qk_proj_pair([('k', 6, 0), ('k', 7, 0)]))
        # mg=1 K units: needed at groups 32+4n; spread through mg=0 phase
        for p in range(4):
            sched(6 + 6 * p, lambda p=p: qk_proj_pair(
                [('k', 2 * p, 1), ('k', 2 * p + 1, 1)]))
        # V units: needed at group 2g (and again at 32+2g)
        for g in range(1, 16):
            sched(2 * g - 2, lambda g=g: v_proj_unit(g))

        # ---- main attention loop: mg outer, 32 k-chunks inner ----
        # per group: 4 scores MMs -> stA (heads 4mg+0,1) + stB (heads 4mg+2,3)
        # ScalarE exps stA into pt[:, :1024] (f32); VectorE Schraudolphs stB
        # into pt[:, 1024:] (int32 bit pattern = f32 exp).  PV/Z matmuls
        # (float32r) accumulate in PSUM across all 32 chunks.
        def pvz_unit(pt, kc, mg, pvacc, zacc):
            for j in range(4):
                h = 4 * mg + j
                nc.tensor.matmul(
                    pvacc[ds(32 * j, 32), :],
                    vt[:, kc, ds(32 * h, 32)],
                    pt[:, ts(j, NQ)],
                    start=(kc == 0), stop=(kc == 31),
                    tile_position=(0, 32 * j))
            for j in range(4):
                nc.tensor.matmul(
                    zacc[ds(32 * j, 1), :],
                    ones[:],
                    pt[:, ts(j, NQ)],
                    start=(kc == 0), stop=(kc == 31),
                    tile_position=(0, 32 * j))

        prev = None
        for mg in range(2):
            pvacc = stp.tile([128, NQ], f32, tag="pv", bufs=1, name=f"pvacc{mg}")
            zacc = stp.tile([128, NQ], f32, tag="z", bufs=1, name=f"zacc{mg}")
            # rows of zacc outside {0,32,64,96} are never written by the PE
            # but flow into the selz gather (x0.0) - keep them finite.
            nc.vector.memset(zacc[:], 0.0)
            for kc in range(32):
                g = 32 * mg + kc
                sth = [stp.tile([128, NQ], f32, tag=f"st{j}", bufs=1,
                                name=f"st{j}") for j in range(4)]
                for j in range(4):
                    nc.tensor.matmul(
                        sth[j][:, :NQ],
                        kt[ds(32 * j, 32), mg, ds(128 * kc, 128)],
                        qt[ds(32 * j, 32), mg, :],
                        start=True, stop=True,
                        tile_position=(32 * j, 0))
                pt = ptp.tile([128, 4 * NQ], bf16, tag="pt", name="pt")
                pti = pt.bitcast(i16)
                nc.scalar.activation(pt[:, ds(0, NQ)], sth[0][:, :NQ], Exp,
                                     scale=SCALE)
                nc.scalar.activation(pt[:, ds(NQ, NQ)], sth[1][:, :NQ], Exp,
                                     scale=SCALE)
                nc.vector.tensor_scalar(pti[:, ds(2 * NQ, NQ)], sth[2][:, :NQ],
                                        A_EXP, B_EXP, mult, add)
                nc.vector.tensor_scalar(pti[:, ds(3 * NQ, NQ)], sth[3][:, :NQ],
                                        A_EXP, B_EXP, mult, add)
                for fn in pre_work.get(g, []):
                    fn()
                if prev is not None:
                    pvz_unit(*prev)
                prev = (pt, kc, mg, pvacc, zacc)
            pvz_unit(*prev)
            prev = None
            # drain this mg's accumulators so the next mg (or epilogue) can
            # reuse the banks; WAR through the tag rings orders everything.
            nc.vector.tensor_copy(out=pvs[:, mg, :], in_=pvacc[:])
            nc.vector.tensor_copy(out=zsb[:, mg, :], in_=zacc[:])
            # per-mg normalization chain: gather Z rows {32j}, reciprocal,
            # broadcast, normalize + V-bias.  mg0's chain hides in the mg1
            # phase; only mg1's chain is an exposed tail.
            zqm = stp.tile([128, NQ], f32, tag="work", bufs=2, name=f"zq{mg}")
            nc.tensor.matmul(zqm[:4, :NQ], selz[:], zsb[:, mg, :],
                             start=True, stop=True)
            nc.vector.reciprocal_approx_fast(zrm[:, mg, :], zqm[:4, :NQ])
            psb = stp.tile([128, NQ], f32, tag="work", bufs=2, name=f"psb{mg}")
            nc.tensor.matmul(psb[:, :NQ], bsel[:], zrm[:4, mg, :],
                             start=True, stop=True)
            nc.vector.tensor_tensor(attn[:, mg, :], pvs[:, mg, :], psb[:, :NQ],
                                    mult)
            nc.vector.tensor_scalar_add(attn[:, mg, :], attn[:, mg, :],
                                        bt[:, 4 * mg + 2:4 * mg + 3])

        # ---- epilogue: final linear (2-wide across the two output halves) ----
        out_r = out.rearrange("(m p) q -> p m q", p=128)
        lins = [stp.tile([128, NQ], f32, tag="work", bufs=2, name=f"lin{mo}")
                for mo in range(2)]
        for mo in range(2):
            for c in range(2):
                nc.tensor.matmul(lins[mo][:, :NQ], lwt[:, c, ts(mo, 128)],
                                 attn[:, c, :], start=(c == 0), stop=(c == 1))
        for mo in range(2):
            nc.vector.tensor_scalar_add(outsb[:, mo, :], lins[mo][:, :NQ],
                                        bt[:, 4 * mo + 3:4 * mo + 4])
            nc.sync.dma_start(out_r[:, mo], outsb[:, mo, :])

    nc.compile()
    return nc


def _get_nc():
    if "nc" not in _CACHE:
        _CACHE["nc"] = _build_nc()
    return _CACHE["nc"]


def _prep_in_maps(input_x, pe_Q, pe_K, WQ, WK, WV, Q_bias, K_bias, V_bias,
                  lin_w, lin_b):
    bf = ml_dtypes.bfloat16
    x_kT = np.ascontiguousarray(
        np.concatenate([input_x, pe_K], axis=1).T.astype(bf))       # [1024, 4096]
    x_q = np.concatenate([input_x, pe_Q], axis=1)                   # [4096, 1024]
    wq2 = np.ascontiguousarray(
        WQ.transpose(1, 0, 2).reshape(QKD, HID).astype(bf))         # [d,(h,hd)]
    wk2 = np.ascontiguousarray(WK.transpose(1, 0, 2).reshape(QKD, HID).astype(bf))
    wv2 = np.ascontiguousarray(WV.transpose(1, 0, 2).reshape(IND, HID).astype(bf))
    lwTn = np.ascontiguousarray(lin_w.T.astype(bf))                 # [in, out]
    bias4 = np.zeros((128, 8), np.float32)
    for m in range(2):
        for i, vec in enumerate([Q_bias.reshape(HID), K_bias.reshape(HID),
                                 V_bias.reshape(HID), lin_b.reshape(HID)]):
            bias4[:, 4 * m + i] = vec[128 * m:128 * (m + 1)]
    in_maps = []
    for i in range(NCORES):
        xqT_i = np.ascontiguousarray(
            x_q[i * NQ:(i + 1) * NQ].T.astype(bf))                  # [1024, 512]
        in_maps.append({
            "xkT": x_kT, "xqT": xqT_i, "wq": wq2, "wk": wk2, "wv": wv2,
            "lwT": lwTn, "bias4": bias4,
        })
    return in_maps


def _ensure_ntff_hook():
    """The agent image's antenv lacks axon_hooks; synthesize it from the
    boot script's ctypes NTFF implementation so trace=True works."""
    import types
    try:
        from antenv.axon_hooks import get_axon_ntff_profile_hook  # noqa: F401
        return
    except ImportError:
        pass
    sys.path.insert(0, "/root/.axon_site/trn_agent_boot")
    import trn_boot
    hook = trn_boot._ntff_profile_via_ctypes(
        os.environ.get("PJRT_LIBRARY_PATH", "/opt/axon/libaxon_pjrt.so"))
    mod = types.ModuleType("antenv.axon_hooks")
    mod._hook = hook
    mod.get_axon_ntff_profile_hook = lambda: mod._hook
    mod.set_axon_ntff_profile_hook = lambda h: setattr(mod, "_hook", h)
    sys.modules["antenv.axon_hooks"] = mod


def _run(in_maps, trace=False):
    from concourse.bass_utils import run_bass_kernel_spmd
    if trace:
        _ensure_ntff_hook()
    nc = _get_nc()
    res = run_bass_kernel_spmd(nc, in_maps, core_ids=list(range(NCORES)),
                               trace=trace)
    return res


def kernel(input_x, pe_Q, pe_K, A, WQ, WK, WV, Q_bias, K_bias, V_bias,
           lin_w, lin_b):
    in_maps = _prep_in_maps(
        np.asarray(input_x, np.float32), np.asarray(pe_Q, np.float32),
        np.asarray(pe_K, np.float32), np.asarray(WQ, np.float32),
        np.asarray(WK, np.float32), np.asarray(WV, np.float32),
        np.asarray(Q_bias, np.float32), np.asarray(K_bias, np.float32),
        np.asarray(V_bias, np.float32), np.asarray(lin_w, np.float32),
        np.asarray(lin_b, np.float32))
    res = _run(in_maps)
    out_full = np.empty((N, HID), np.float32)
    for i in range(NCORES):
        out_full[i * NQ:(i + 1) * NQ] = res.results[i]["out"].T
    return out_full


def hw_exec_ns(input_x, pe_Q, pe_K, A, WQ, WK, WV, Q_bias, K_bias, V_bias,
               lin_w, lin_b):
    """Run once with NTFF tracing; returns (exec_time_ns, results)."""
    in_maps = _prep_in_maps(
        np.asarray(input_x, np.float32), np.asarray(pe_Q, np.float32),
        np.asarray(pe_K, np.float32), np.asarray(WQ, np.float32),
        np.asarray(WK, np.float32), np.asarray(WV, np.float32),
        np.asarray(Q_bias, np.float32), np.asarray(K_bias, np.float32),
        np.asarray(V_bias, np.float32), np.asarray(lin_w, np.float32),
        np.asarray(lin_b, np.float32))
    res = _run(in_maps, trace=True)
    return res.exec_time_ns, res


# revision 17
# speedup vs baseline: 1.2735x; 1.2735x over previous
"""Trainium2 Bass kernel for nn_Attention_layer (GNN message passing attention).

Math (see harness reference):
  x_Q = [input_x, pe_Q]  (N, 1024);  x_K = [input_x, pe_K]
  Q = x_Q @ WQ[h] + qb;  K = x_K @ WK[h] + kb;  V = input_x @ WV[h] + vb
  attn = softmax(Q K^T / 16, axis=k);  out = concat_h(attn @ V) @ lin_w.T + lin_b

Distribution: 8 NeuronCores, query-dim (N) sharded 512 rows/core; K/V work
replicated (no collectives).  Everything is computed in the transposed domain
(scores^T with k-nodes on partitions) so no on-device transposes are needed.

v2+ design (from perfetto analysis of v1, which was ScalarE-bound at ~80%
occupancy on 16.8M exp elements/core; v1 = 180-211us, this = ~172us in the
same device state):
  - exp is SPLIT between ScalarE (exact, heads 4m+{0,1}) and VectorE
    (Schraudolph bit-trick exp, heads 4m+{2,3}): one tensor_scalar mult-add
    per tile whose f32->int16 convert emits exactly the BF16 bit pattern of
    exp(s/16).  End-to-end rel err 9.8e-3 (gate 2e-2).
  - scores live in 4 one-bank PSUM tiles (tags st0..st3), one per head, so
    each exp call releases its bank independently (finer WAR pipelining;
    2 ACT calls + 2 DVE calls per group).
  - PV and Z (ones-row) matmuls accumulate IN PSUM across all 32 k-chunks
    (start/stop chains) instead of VectorE adds - frees ~106us/core of DVE.
  - head-group-OUTER loop (mg=0 fully, then mg=1) so only one PV + one Z
    accumulator bank is live at a time; PSUM budget: 4x scores + pv + z +
    2x proj work ring = 8 banks exactly.  Per-mg normalization chain runs
    right after each mg's drain (mg0's hides inside the mg1 phase).
  - projections (Q/K/V) interleaved into the main loop via the work ring,
    paced a bit ahead of the attention groups that consume them; input DMA
    split across BOTH hwdge queues (sync + scalar) to double head bandwidth.

Measured laws this design is built around (this hw):
  - a 4-way tiled matmul group (row- or col-tiled) streams ~2 cols/cycle
    aggregate; serial full-array MMs get ~1 col/cycle (216ns per N=512).
  - ACTIVATE [128,512] PSUM->SBUF ~687ns; VectorE tensor_scalar ~682ns.
  - do NOT interleave MMs of two full-array accumulation chains (alternating
    LDWEIGHTS at the same tile position corrupts results silently).
  - fp8 (e4m3) projections fail the 2e-2 gate (measured 4-5e-2 end to end).
"""

import os
import sys
import numpy as np
import ml_dtypes

for _p in ("/opt/trn_rl_repo", "/root/.axon_site/_ro/trn_rl_repo"):
    if os.path.isdir(_p) and _p not in sys.path:
        sys.path.insert(0, _p)

N = 4096
IND = 256          # input_x dim
QKD = 1024         # concat dim for Q/K projections
H = 8              # heads
HD = 32            # head dim
HID = 256          # H * HD
NCORES = 8
NQ = N // NCORES   # 512 query rows per core
SCALE = 1.0 / 16.0  # 1/sqrt(HID)

# Schraudolph exp constants (see numpy calibration): the top 16 bits of the
# f32 pattern of exp(s/16) are int16(s * A_EXP + B_EXP) -- i.e. one VectorE
# tensor_scalar writing int16 produces the BF16 of exp.  C=366000 minimizes
# max rel err (~3.3%).
A_EXP = float((2.0 ** 23) * np.log2(np.e) / 16.0 / 65536.0)
B_EXP = float((127.0 * 2.0 ** 23 - 366000.0) / 65536.0)

_CACHE = {}


def _build_nc():
    from contextlib import ExitStack
    import concourse.bacc as bacc
    import concourse.tile as tile
    import concourse.mybir as mybir
    from concourse.bass import ds, ts

    f32 = mybir.dt.float32
    i16 = mybir.dt.int16
    bf16 = mybir.dt.bfloat16
    Exp = mybir.ActivationFunctionType.Exp
    Ident = mybir.ActivationFunctionType.Identity
    Copy = mybir.ActivationFunctionType.Copy
    mult = mybir.AluOpType.mult
    add = mybir.AluOpType.add

    nc = bacc.Bacc("TRN2", target_bir_lowering=False, debug=False,
                   num_devices=NCORES)

    # ---- DRAM I/O (per-core shards prepared on host) ----
    xkT = nc.dram_tensor("xkT", [QKD, N], bf16, kind="ExternalInput")   # [x;peK]^T
    xqT = nc.dram_tensor("xqT", [QKD, NQ], bf16, kind="ExternalInput")  # [x;peQ]^T rows blk
    wq = nc.dram_tensor("wq", [QKD, HID], bf16, kind="ExternalInput")   # [d,(h,hd)]
    wk = nc.dram_tensor("wk", [QKD, HID], bf16, kind="ExternalInput")
    wv = nc.dram_tensor("wv", [IND, HID], bf16, kind="ExternalInput")
    lwT = nc.dram_tensor("lwT", [HID, HID], bf16, kind="ExternalInput")  # lin_w.T
    bias4 = nc.dram_tensor("bias4", [128, 8], f32, kind="ExternalInput")  # [p, 4m+i]
    out = nc.dram_tensor("out", [HID, NQ], f32, kind="ExternalOutput")   # out^T

    # Z-row gather: zacc drains hold Z_{4mg+r} at partition 32r; gather to
    # rows 0..3 (mg-independent).
    selz_np = np.zeros((128, 4), dtype=np.float32)
    for r in range(4):
        selz_np[32 * r, r] = 1.0
    selz_dram = nc.inline_tensor(np.ascontiguousarray(selz_np), name="selz_const")
    # 1/Z broadcast: psb[32j+hd, q] = zrm[j, q]  (mg-independent)
    bsel_np = np.zeros((4, 128), dtype=np.float32)
    for j in range(4):
        bsel_np[j, 32 * j:32 * j + 32] = 1.0
    bsel_dram = nc.inline_tensor(bsel_np, name="bsel_const")
    ones_np = np.ones((128, 1), dtype=ml_dtypes.bfloat16)
    ones_dram = nc.inline_tensor(ones_np, name="ones_const")

    with tile.TileContext(nc) as tc, ExitStack() as ctx:
        consts = ctx.enter_context(tc.tile_pool(name="consts", bufs=1))
        big = ctx.enter_context(tc.tile_pool(name="big", bufs=1))
        ptp = ctx.enter_context(tc.tile_pool(name="ptp", bufs=3))
        stp = ctx.enter_context(tc.tile_pool(name="stp", bufs=1, space="PSUM"))

        # ---- SBUF tiles ----
        xkt = big.tile([128, 8, N], bf16, tag="xkt")       # x_K^T  (8 c-chunks)
        xqt = big.tile([128, 8, NQ], bf16, tag="xqt")      # x_Q^T block
        wqt = consts.tile([128, 8, HID], bf16, tag="wqt")
        wkt = consts.tile([128, 8, HID], bf16, tag="wkt")
        wvt = consts.tile([128, 2, HID], bf16, tag="wvt")
        lwt = consts.tile([128, 2, HID], bf16, tag="lwt")
        bt = consts.tile([128, 8], f32, tag="bt")          # [p, 4m+i]
        selz = consts.tile([128, 4], f32, tag="selz")
        bsel = consts.tile([4, 128], f32, tag="bsel")
        ones = consts.tile([128, 1], bf16, tag="ones")

        kt = big.tile([128, 2, N], bf16, tag="kt")         # K^T rows (h,hd)
        qt = big.tile([128, 2, NQ], bf16, tag="qt")        # Q^T
        vt = big.tile([128, 32, HID], bf16, tag="vt")      # V node-major
        pvs = big.tile([128, 2, NQ], f32, tag="pvs")       # PV accum drains
        zsb = big.tile([128, 2, NQ], f32, tag="zsb")       # Z accum drains
        zrm = big.tile([4, 2, NQ], f32, tag="zrm")         # 1/Z per head (mg slot)
        attn = big.tile([128, 2, NQ], bf16, tag="attn")    # normalized attn_x^T
        outsb = big.tile([128, 2, NQ], f32, tag="outsb")

        # ---- const / weight DMAs, ordered by first consumer ----
        xkT_r = xkT.rearrange("(c p) (n q) -> n p c q", p=128, q=512)
        xqT_r = xqT.rearrange("(c p) q -> p c q", p=128)
        nc.sync.dma_start(wqt[:], wq.rearrange("(c p) o -> p c o", p=128))
        nc.sync.dma_start(bt[:], bias4[:])
        nc.scalar.dma_start(xqt[:, :4], xqT_r[:, :4])
        nc.scalar.dma_start(xqt[:, 4:], xqT_r[:, 4:])
        nc.sync.dma_start(wkt[:], wk.rearrange("(c p) o -> p c o", p=128))
        nc.sync.dma_start(wvt[:], wv.rearrange("(c p) o -> p c o", p=128))
        nc.scalar.dma_start(xkt[:, :, ds(0, 128)], xkT_r[0][:, :, ds(0, 128)])
        nc.sync.dma_start(xkt[:, :, ds(128, 384)], xkT_r[0][:, :, ds(128, 384)])
        nc.sync.dma_start(lwt[:], lwT.rearrange("(c p) o -> p c o", p=128))
        nc.sync.dma_start(selz[:], selz_dram[:])
        nc.sync.dma_start(bsel[:], bsel_dram[:])
        nc.sync.dma_start(ones[:], ones_dram[:])
        # dual-queue: scalar's HWDGE queue carries half of each node chunk
        # concurrently with sync's half
        for n in range(1, 8):
            nc.scalar.dma_start(xkt[:, :4, ts(n, 512)], xkT_r[n][:, :4])
            nc.sync.dma_start(xkt[:, 4:, ts(n, 512)], xkT_r[n][:, 4:])

        # preload the ACT exp table set while DMAs land
        actwarm = consts.tile([8, 16], f32, tag="actwarm")
        nc.vector.memset(actwarm[:], 0.0)
        nc.scalar.activation(actwarm[:], actwarm[:], Exp)

        # ---- projection units (PSUM work ring, 2 banks) ----
        # Two independent half-units interleave their MMs so the PE streams
        # 2-wide (adjacent MMs hit different banks; serial same-bank chains
        # only reach 1 col/cycle).
        def qk_proj_pair(specs):
            # specs: list of 2 (kind, n, m) where kind in 'q','k'
            tiles = []
            for kind, n, m in specs:
                ps = stp.tile([128, NQ], f32, tag="work", bufs=2,
                              name=f"{kind}p{n}_{m}")
                tiles.append(ps)
            for (kind, n, m), ps in zip(specs, tiles):
                for c in range(8):
                    if kind == 'q':
                        nc.tensor.matmul(ps[:, :NQ], wqt[:, c, ts(m, 128)],
                                         xqt[:, c, :],
                                         start=(c == 0), stop=(c == 7))
                    else:
                        nc.tensor.matmul(ps[:, :NQ], wkt[:, c, ts(m, 128)],
                                         xkt[:, c, ts(n, 512)],
                                         start=(c == 0), stop=(c == 7))
            for (kind, n, m), ps in zip(specs, tiles):
                if kind == 'q':
                    nc.vector.tensor_scalar_add(qt[:, m, :], ps[:, :NQ],
                                                bt[:, 4 * m + 0:4 * m + 1])
                else:
                    nc.vector.tensor_scalar_add(kt[:, m, ts(n, 512)], ps[:, :NQ],
                                                bt[:, 4 * m + 1:4 * m + 2])

        def k_proj_unit(n, m, lo=0, w=512):
            ps = stp.tile([128, NQ], f32, tag="work", bufs=2, name=f"kp{n}_{m}")
            for c in range(8):
                nc.tensor.matmul(ps[:, :w], wkt[:, c, ts(m, 128)],
                                 xkt[:, c, ds(512 * n + lo, w)],
                                 start=(c == 0), stop=(c == 7))
            nc.vector.tensor_scalar_add(kt[:, m, ds(512 * n + lo, w)], ps[:, :w],
                                        bt[:, 4 * m + 1:4 * m + 2])

        def v_proj_unit(g):
            # covers node chunks 2g, 2g+1 -> vt[:, 2g:2g+2, :]; the two
            # chunks' accumulations interleave for 2-wide streaming.
            ps = stp.tile([128, NQ], f32, tag="work", bufs=2, name=f"vp{g}")
            for kc in (2 * g, 2 * g + 1):
                off = 256 * (kc - 2 * g)
                for c in range(2):
                    nc.tensor.matmul(ps[:, ds(off, HID)],
                                     xkt[:, c, ds(128 * kc, 128)],
                                     wvt[:, c, :], start=(c == 0), stop=(c == 1))
            nc.vector.tensor_copy(out=vt[:, 2 * g:2 * g + 2, :],
                                  in_=ps[:].rearrange("p (g o) -> p g o", o=HID))

        # ---- prologue: minimum for attention group (kc=0, mg=0) ----
        qk_proj_pair([('q', 0, 0), ('k', 0, 0)])
        v_proj_unit(0)

        # remaining proj work, scheduled into the group stream.
        # group index g = 32*mg + kc  (mg outer).  A k_proj_unit(n, m) must
        # complete before group (32*m + 4*n); v_proj_unit(g) before groups
        # (mg, 2g) i.e. min(2g, 32+2g)=2g for mg=0... v needed by both mgs at
        # kc=2g -> before group 2g.
        pre_work = {}

        def sched(slot, fn):
            pre_work.setdefault(max(0, slot), []).append(fn)

        # mg=0 K units paired with q(1)/each other; needed at groups 4n.
        sched(0, lambda: qk_proj_pair([('k', 1, 0), ('q', 0, 1)]))
        sched(3, lambda: qk_proj_pair([('k', 2, 0), ('k', 3, 0)]))
        sched(9, lambda: qk_proj_pair([('k', 4, 0), ('k', 5, 0)]))
        sched(17, lambda: qk_proj_pair([('k', 6, 0), ('k', 7, 0)]))
        # mg=1 K units: needed at groups 32+4n; spread through mg=0 phase
        for p in range(4):
            sched(6 + 6 * p, lambda p=p: qk_proj_pair(
                [('k', 2 * p, 1), ('k', 2 * p + 1, 1)]))
        # V units: needed at group 2g (and again at 32+2g)
        for g in range(1, 16):
            sched(2 * g - 2, lambda g=g: v_proj_unit(g))

        # ---- main attention loop: mg outer, 32 k-chunks inner ----
        # per group: 4 scores MMs -> stA (heads 4mg+0,1) + stB (heads 4mg+2,3)
        # ScalarE exps stA into pt[:, :1024] (f32); VectorE Schraudolphs stB
        # into pt[:, 1024:] (int32 bit pattern = f32 exp).  PV/Z matmuls
        # (float32r) accumulate in PSUM across all 32 chunks.
        def pvz_unit(pt, kc, mg, pvacc, zacc):
            for j in range(4):
                h = 4 * mg + j
                nc.tensor.matmul(
                    pvacc[ds(32 * j, 32), :],
                    vt[:, kc, ds(32 * h, 32)],
                    pt[:, ts(j, NQ)],
                    start=(kc == 0), stop=(kc == 31),
                    tile_position=(0, 32 * j))
            for j in range(4):
                nc.tensor.matmul(
                    zacc[ds(32 * j, 1), :],
                    ones[:],
                    pt[:, ts(j, NQ)],
                    start=(kc == 0), stop=(kc == 31),
                    tile_position=(0, 32 * j))

        prev = None
        for mg in range(2):
            pvacc = stp.tile([128, NQ], f32, tag="pv", bufs=1, name=f"pvacc{mg}")
            zacc = stp.tile([128, NQ], f32, tag="z", bufs=1, name=f"zacc{mg}")
            # rows of zacc outside {0,32,64,96} are never written by the PE
            # but flow into the selz gather (x0.0) - keep them finite.
            nc.vector.memset(zacc[:], 0.0)
            for kc in range(32):
                g = 32 * mg + kc
                sth = [stp.tile([128, NQ], f32, tag=f"st{j}", bufs=1,
                                name=f"st{j}") for j in range(4)]
                for j in range(4):
                    nc.tensor.matmul(
                        sth[j][:, :NQ],
                        kt[ds(32 * j, 32), mg, ds(128 * kc, 128)],
                        qt[ds(32 * j, 32), mg, :],
                        start=True, stop=True,
                        tile_position=(32 * j, 0))
                pt = ptp.tile([128, 4 * NQ], bf16, tag="pt", name="pt")
                pti = pt.bitcast(i16)
                nc.scalar.activation(pt[:, ds(0, NQ)], sth[0][:, :NQ], Exp,
                                     scale=SCALE)
                nc.scalar.activation(pt[:, ds(NQ, NQ)], sth[1][:, :NQ], Exp,
                                     scale=SCALE)
                nc.vector.tensor_scalar(pti[:, ds(2 * NQ, NQ)], sth[2][:, :NQ],
                                        A_EXP, B_EXP, mult, add)
                nc.vector.tensor_scalar(pti[:, ds(3 * NQ, NQ)], sth[3][:, :NQ],
                                        A_EXP, B_EXP, mult, add)
                for fn in pre_work.get(g, []):
                    fn()
                if prev is not None:
                    pvz_unit(*prev)
                prev = (pt, kc, mg, pvacc, zacc)
            pvz_unit(*prev)
            prev = None
            # drain this mg's accumulators so the next mg (or epilogue) can
            # reuse the banks; WAR through the tag rings orders everything.
            nc.vector.tensor_copy(out=pvs[:, mg, :], in_=pvacc[:])
            nc.vector.tensor_copy(out=zsb[:, mg, :], in_=zacc[:])
            # per-mg normalization chain: gather Z rows {32j}, reciprocal,
            # broadcast, normalize + V-bias.  mg0's chain hides in the mg1
            # phase; only mg1's chain is an exposed tail.
            zqm = stp.tile([128, NQ], f32, tag="work", bufs=2, name=f"zq{mg}")
            nc.tensor.matmul(zqm[:4, :NQ], selz[:], zsb[:, mg, :],
                             start=True, stop=True)
            nc.vector.reciprocal_approx_fast(zrm[:, mg, :], zqm[:4, :NQ])
            psb = stp.tile([128, NQ], f32, tag="work", bufs=2, name=f"psb{mg}")
            nc.tensor.matmul(psb[:, :NQ], bsel[:], zrm[:4, mg, :],
                             start=True, stop=True)
            nc.vector.tensor_tensor(attn[:, mg, :], pvs[:, mg, :], psb[:, :NQ],
                                    mult)
            nc.vector.tensor_scalar_add(attn[:, mg, :], attn[:, mg, :],
                                        bt[:, 4 * mg + 2:4 * mg + 3])

        # ---- epilogue: final linear (2-wide across the two output halves) ----
        out_r = out.rearrange("(m p) q -> p m q", p=128)
        lins = [stp.tile([128, NQ], f32, tag="work", bufs=2, name=f"lin{mo}")
                for mo in range(2)]
        for mo in range(2):
            for c in range(2):
                nc.tensor.matmul(lins[mo][:, :NQ], lwt[:, c, ts(mo, 128)],
                                 attn[:, c, :], start=(c == 0), stop=(c == 1))
        for mo in range(2):
            nc.vector.tensor_scalar_add(outsb[:, mo, :], lins[mo][:, :NQ],
                                        bt[:, 4 * mo + 3:4 * mo + 4])
            nc.sync.dma_start(out_r[:, mo], outsb[:, mo, :])

    nc.compile()
    return nc


def _get_nc():
    if "nc" not in _CACHE:
        _CACHE["nc"] = _build_nc()
    return _CACHE["nc"]


def _prep_in_maps(input_x, pe_Q, pe_K, WQ, WK, WV, Q_bias, K_bias, V_bias,
                  lin_w, lin_b):
    bf = ml_dtypes.bfloat16
    x_kT = np.ascontiguousarray(
        np.concatenate([input_x, pe_K], axis=1).T.astype(bf))       # [1024, 4096]
    x_q = np.concatenate([input_x, pe_Q], axis=1)                   # [4096, 1024]
    wq2 = np.ascontiguousarray(
        WQ.transpose(1, 0, 2).reshape(QKD, HID).astype(bf))         # [d,(h,hd)]
    wk2 = np.ascontiguousarray(WK.transpose(1, 0, 2).reshape(QKD, HID).astype(bf))
    wv2 = np.ascontiguousarray(WV.transpose(1, 0, 2).reshape(IND, HID).astype(bf))
    lwTn = np.ascontiguousarray(lin_w.T.astype(bf))                 # [in, out]
    bias4 = np.zeros((128, 8), np.float32)
    for m in range(2):
        for i, vec in enumerate([Q_bias.reshape(HID), K_bias.reshape(HID),
                                 V_bias.reshape(HID), lin_b.reshape(HID)]):
            bias4[:, 4 * m + i] = vec[128 * m:128 * (m + 1)]
    in_maps = []
    for i in range(NCORES):
        xqT_i = np.ascontiguousarray(
            x_q[i * NQ:(i + 1) * NQ].T.astype(bf))                  # [1024, 512]
        in_maps.append({
            "xkT": x_kT, "xqT": xqT_i, "wq": wq2, "wk": wk2, "wv": wv2,
            "lwT": lwTn, "bias4": bias4,
        })
    return in_maps


def _ensure_ntff_hook():
    """The agent image's antenv lacks axon_hooks; synthesize it from the
    boot script's ctypes NTFF implementation so trace=True works."""
    import types
    try:
        from antenv.axon_hooks import get_axon_ntff_profile_hook  # noqa: F401
        return
    except ImportError:
        pass
    sys.path.insert(0, "/root/.axon_site/trn_agent_boot")
    import trn_boot
    hook = trn_boot._ntff_profile_via_ctypes(
        os.environ.get("PJRT_LIBRARY_PATH", "/opt/axon/libaxon_pjrt.so"))
    mod = types.ModuleType("antenv.axon_hooks")
    mod._hook = hook
    mod.get_axon_ntff_profile_hook = lambda: mod._hook
    mod.set_axon_ntff_profile_hook = lambda h: setattr(mod, "_hook", h)
    sys.modules["antenv.axon_hooks"] = mod


def _run(in_maps, trace=False):
    from concourse.bass_utils import run_bass_kernel_spmd
    if trace:
        _ensure_ntff_hook()
    nc = _get_nc()
    res = run_bass_kernel_spmd(nc, in_maps, core_ids=list(range(NCORES)),
                               trace=trace)
    return res


def kernel(input_x, pe_Q, pe_K, A, WQ, WK, WV, Q_bias, K_bias, V_bias,
           lin_w, lin_b):
    in_maps = _prep_in_maps(
        np.asarray(input_x, np.float32), np.asarray(pe_Q, np.float32),
        np.asarray(pe_K, np.float32), np.asarray(WQ, np.float32),
        np.asarray(WK, np.float32), np.asarray(WV, np.float32),
        np.asarray(Q_bias, np.float32), np.asarray(K_bias, np.float32),
        np.asarray(V_bias, np.float32), np.asarray(lin_w, np.float32),
        np.asarray(lin_b, np.float32))
    res = _run(in_maps)
    out_full = np.empty((N, HID), np.float32)
    for i in range(NCORES):
        out_full[i * NQ:(i + 1) * NQ] = res.results[i]["out"].T
    return out_full


def hw_exec_ns(input_x, pe_Q, pe_K, A, WQ, WK, WV, Q_bias, K_bias, V_bias,
               lin_w, lin_b):
    """Run once with NTFF tracing; returns (exec_time_ns, results)."""
    in_maps = _prep_in_maps(
        np.asarray(input_x, np.float32), np.asarray(pe_Q, np.float32),
        np.asarray(pe_K, np.float32), np.asarray(WQ, np.float32),
        np.asarray(WK, np.float32), np.asarray(WV, np.float32),
        np.asarray(Q_bias, np.float32), np.asarray(K_bias, np.float32),
        np.asarray(V_bias, np.float32), np.asarray(lin_w, np.float32),
        np.asarray(lin_b, np.float32))
    res = _run(in_maps, trace=True)
    return res.exec_time_ns, res
